# revision 1
# baseline (speedup 1.0000x reference)
"""Trainium2 Bass kernel for nn_Loss_20495583936604 (pairwise BCE ranking loss).

Reference semantics: over all pairs i<j with b[i]==b[j] and y[i]!=y[j],
mean of BCE-with-logits(d = s[i]-s[j], target z = (y[i]==1)).

Math reduction
--------------
Every valid unordered pair has exactly one positive (y==1) and one negative
(y==0) element, and its BCE term equals softplus(s_neg - s_pos) regardless of
index order.  So with segments g and P = sum_g |neg(g)|*|pos(g)| pairs:

    loss = (1/P) * sum_g sum_{n in neg(g)} sum_{p in pos(g)}
                       log(1 + exp(s_n) * exp(-s_p))

Host side does O(N) layout only: per segment, pack -s_pos into a [128, wp]
tile and s_neg into [128, wn] (partition = segment; NUM_SEGMENTS == 128),
padding with -1e4 so padded slots exp() to exactly 0 and contribute
log(1+0) = 0.

Device side (one NeuronCore program, SPMD over 8 cores; cores split the
wn neg-slots — a data-parallel shard of the pair-matrix rows):
    1. one DMA brings in [-s_pos | s_neg-slice]            (sync, HW DGE)
    2. e = exp(input)  - one ACT pass over both halves     (scalar)
    3. d = e_neg (x) e_pos outer product per partition via
       zero-stride broadcast APs - one DVE tensor_tensor   (vector)
    4. softplus = ln(d + 1) with free-dim accumulation     (scalar)
    5. partition reduce: ones^T @ acc matmul -> PSUM[1,1]  (tensor)
    6. PSUM -> SBUF copy, then a single-descriptor DMA out (vector+sync)
Host sums the 8 partial sums and divides by the (host-counted) pair count.

Perf notes baked in:
  * a dummy exp on a [1,1] tile hoists the ~1.3us ACT_TABLE_LOAD into the
    input-DMA latency shadow;
  * the ln table load overlaps the DVE multiply;
  * the output is reduced to [1,1] on-chip because a [128,1] store sprays
    128 4-byte descriptors over 16 DMA queues whose per-queue semaphore
    increments straggle in over ~5us;
  * the Bass-init all-engine barrier is narrowed to {gpsimd, scalar} (the
    const-AP producer/consumer pair) so nobody waits ~3us for the PE
    engine's cold boot;
  * the kernel ends with barrier + semaphore clear so the core is left
    clean for the next NEFF (omitting this wedges the device).
"""

import sys

if "/opt/trn_rl_repo" not in sys.path:
    sys.path.insert(0, "/opt/trn_rl_repo")

import numpy as np

import concourse.bass as bass
from concourse import bacc, mybir
from concourse.bass_utils import run_bass_kernel_spmd

N_CORES = 8
N_PART = 128
PAD = -1.0e4  # exp(PAD) == 0.0 in f32
SCORE_RANGE_LIMIT = 25.0  # |s_i - s_j| beyond this risks exp/ln range issues

_program_cache: dict[tuple[int, int], "bacc.Bacc"] = {}


def _build_program(wp: int, k: int) -> "bacc.Bacc":
    f32 = mybir.dt.float32
    w_tot = wp + k

    # Stock Bass.__init__ ends with an ALL-engine barrier guarding the
    # const-AP memsets (gpsimd writes, scalar reads the Ln bias constant).
    # Only Pool -> Activation ordering is needed; the full barrier makes
    # every engine wait ~3us for the PE engine's cold boot.
    # Of the four const APs Bass.__init__ memsets, only f32 1.0 (the Ln
    # bias) is ever read here; skipping the rest lets the init barrier
    # release the scalar engine a few hundred ns sooner.
    orig_memset = bass.BassGpSimd.memset

    def sparse_const_memset(self, ap, value, *args, **kwargs):
        name = getattr(ap.tensor, "name", "")
        if name.startswith("const-") and name != "const-float32-1.0":
            return None
        return orig_memset(self, ap, value, *args, **kwargs)

    bass.BassGpSimd.memset = sparse_const_memset
    try:
        nc = bacc.Bacc(
            "TRN2", target_bir_lowering=False, debug=False, enable_asserts=False
        )
    finally:
        bass.BassGpSimd.memset = orig_memset

    inp = nc.dram_tensor("inp", [N_PART, w_tot], f32, kind="ExternalInput")
    acc = nc.dram_tensor("acc", [1, 1], f32, kind="ExternalOutput")

    dma_sem = nc.alloc_semaphore("dma_sem")
    s_sem = nc.alloc_semaphore("s_sem")
    v_sem = nc.alloc_semaphore("v_sem")
    g_sem = nc.alloc_semaphore("g_sem")
    t_sem = nc.alloc_semaphore("t_sem")
    c_sem = nc.alloc_semaphore("c_sem")
    all_sems = [dma_sem, s_sem, v_sem, g_sem, t_sem, c_sem]

    # A previous NEFF (e.g. arbitrary jax ops) may leave semaphores
    # nonzero -- waits would then pass before their producers ran and the
    # kernel reads garbage.  Same protocol stock Bass uses for
    # target_bir_lowering: clear the whole kernel sem range, then the
    # NRT pseudo barrier (valid even while bass sems are untrusted).
    from concourse.bass import compact_to_ranges

    for rng in compact_to_ranges(
        [sh for sh in nc._kernel_sem_range if sh not in nc.barrier_sems]
    ):
        nc.gpsimd.dma_reset(rng)
        nc.gpsimd.sem_clear(rng)
    nc._nrt_pseudo_barrier()

    with (
        nc.sbuf_tensor("in_t", [N_PART, w_tot], f32) as in_t,
        nc.sbuf_tensor("e_t", [N_PART, w_tot], f32) as e_t,
        nc.sbuf_tensor("d_t", [N_PART, k * wp], f32) as d_t,
        nc.sbuf_tensor("sp_t", [N_PART, k * wp], f32) as sp_t,
        nc.sbuf_tensor("acc_t", [N_PART, 1], f32) as acc_t,
        nc.sbuf_tensor("ones_t", [N_PART, 1], f32) as ones_t,
        nc.sbuf_tensor("red_t", [1, 1], f32) as red_t,
        nc.psum_tensor("psum_t", [1, 1], f32) as psum_t,
        nc.sbuf_tensor("dummy_t", [1, 1], f32) as dummy_t,
    ):
        e_ap = e_t.ap()
        a_neg = e_ap[:, wp : wp + k].unsqueeze(-1).broadcast_to([N_PART, k, wp])
        b_pos = e_ap[:, 0:wp].unsqueeze(1).broadcast_to([N_PART, k, wp])
        d3 = d_t.ap().rearrange("p (k w) -> p k w", k=k)

        # input load (HW DGE)
        nc.sync.dma_start(in_t[:], inp.ap()).then_inc(dma_sem, 16)

        # dummy exp: walrus emits the ACT_TABLE_LOAD before it, i.e. inside
        # the DMA-latency shadow, so the real exp finds the table resident.
        nc.gpsimd.memset(dummy_t[:], 1.0)
        nc.gpsimd.memset(ones_t[:], 1.0).then_inc(g_sem, 1)
        nc.scalar.activation(dummy_t[:], dummy_t[:], mybir.ActivationFunctionType.Exp)

        # e = exp(in): exp(-s_pos) | exp(s_neg) in one pass
        nc.scalar.wait_ge(dma_sem, 16)
        nc.scalar.activation(
            e_t[:], in_t[:], mybir.ActivationFunctionType.Exp
        ).then_inc(s_sem, 1)

        # all pairwise products exp(s_n)*exp(-s_p) via zero-stride broadcasts
        nc.vector.wait_ge(s_sem, 1)
        nc.vector.tensor_tensor(d3, a_neg, b_pos, op=mybir.AluOpType.mult).then_inc(
            v_sem, 1
        )

        # softplus = ln(d + 1), accumulated along the free dim (the ln
        # table load this triggers overlaps the DVE multiply above)
        nc.scalar.wait_ge(v_sem, 1)
        nc.scalar.activation(
            sp_t[:],
            d_t[:],
            mybir.ActivationFunctionType.Ln,
            bias=1.0,
            accum_out=acc_t[:],
        ).then_inc(s_sem, 1)

        # partition reduce on PE: psum[1,1] = acc^T @ ones
        nc.tensor.wait_ge(s_sem, 2)
        nc.tensor.wait_ge(g_sem, 1)
        nc.tensor.matmul(
            psum_t[:], acc_t[:], ones_t[:], start=True, stop=True
        ).then_inc(t_sem, 1)

        # PSUM -> SBUF on the idle vector engine, then one [1,1] descriptor
        nc.vector.wait_ge(t_sem, 1)
        nc.vector.tensor_copy(red_t[:], psum_t[:]).then_inc(c_sem, 1)

        nc.sync.wait_ge(c_sem, 1)
        nc.sync.dma_start(acc.ap(), red_t[:]).then_inc(dma_sem, 16)
        nc.sync.wait_ge(dma_sem, 32)

    # leave the core clean: sem-only barrier (dma_sem>=32 above already
    # confirmed every DMA completed, so the per-engine DRAINs of the full
    # barrier are redundant), then gpsimd zeroes the semaphores and halts.
    # (Moving the dma wait onto the gpsimd leader to overlap the handshake
    # was measured 2us SLOWER - the +16 lands faster with sync waiting.)
    nc.all_engine_barrier(sem_only=True)
    nc.clear_and_free_semaphores(all_sems)

    nc.compile()
    return nc


def pack(seg_ids, scores, width, pad):
    """Pack per-segment values into a [128, width] tile, pad-filled."""
    out = np.full((N_PART, width), pad, dtype=np.float32)
    order = np.argsort(seg_ids, kind="stable")
    sorted_seg = seg_ids[order]
    sorted_scores = scores[order]
    counts = np.bincount(sorted_seg, minlength=N_PART)
    starts = np.concatenate([[0], np.cumsum(counts)[:-1]])
    slot = np.arange(len(sorted_seg)) - starts[sorted_seg]
    out[sorted_seg, slot] = sorted_scores
    return out


def make_in_maps(b, s, y):
    seg = np.asarray(b).astype(np.int64)
    s = np.asarray(s, dtype=np.float32)
    is_pos = np.asarray(y) == 1
    cn = np.bincount(seg[~is_pos], minlength=N_PART).astype(np.int64)
    cp = np.bincount(seg[is_pos], minlength=N_PART).astype(np.int64)
    num_pairs = int((cn * cp).sum())
    if num_pairs == 0:
        return None, 0, 0, 0
    wn = int(-(-int(cn.max()) // N_CORES) * N_CORES)  # round up to 8 slots
    wp = int(cp.max())
    k = wn // N_CORES
    sn_packed = pack(seg[~is_pos], s[~is_pos], wn, PAD)
    nsp_packed = pack(seg[is_pos], -s[is_pos], wp, PAD)
    in_maps = [
        {
            "inp": np.ascontiguousarray(
                np.concatenate([nsp_packed, sn_packed[:, c * k : (c + 1) * k]], axis=1)
            )
        }
        for c in range(N_CORES)
    ]
    return in_maps, num_pairs, wp, k


def _host_reference(seg, s, is_pos, num_pairs):
    """Exact fallback for inputs outside the device kernel's numeric
    envelope (never taken for the intended score distribution)."""
    total = 0.0
    for g in range(int(seg.max()) + 1):
        sn = s[(seg == g) & ~is_pos].astype(np.float64)
        sp = s[(seg == g) & is_pos].astype(np.float64)
        if len(sn) and len(sp):
            d = sn[:, None] - sp[None, :]
            total += np.logaddexp(0.0, d).sum()
    return np.float32(total / num_pairs)


def kernel(b: np.ndarray, s: np.ndarray, y: np.ndarray) -> np.ndarray:
    seg = np.asarray(b).astype(np.int64)
    s = np.asarray(s, dtype=np.float32)
    is_pos = np.asarray(y) == 1
    assert seg.min() >= 0 and seg.max() < N_PART, "segment ids must fit 128 partitions"

    in_maps, num_pairs, wp, k = make_in_maps(b, s, y)
    if num_pairs == 0:
        return np.float32(np.nan)
    if float(s.max()) - float(s.min()) > SCORE_RANGE_LIMIT:
        return _host_reference(seg, s, is_pos, num_pairs)

    key = (wp, k)
    nc = _program_cache.get(key)
    if nc is None:
        nc = _build_program(wp, k)
        _program_cache[key] = nc

    results = run_bass_kernel_spmd(nc, in_maps, core_ids=list(range(N_CORES))).results
    total = sum(np.float64(r["acc"][0, 0]) for r in results)
    if not np.isfinite(total):
        # device state was poisoned by a prior NEFF -- fall back to exact host math
        return _host_reference(seg, s, is_pos, num_pairs)
    return np.asarray(total / num_pairs, dtype=np.float32)


if __name__ == "__main__":
    rng = np.random.default_rng(0)
    n = 8192
    b = rng.integers(0, 128, size=n).astype(np.int32)
    s = rng.standard_normal(n).astype(np.float32)
    y = rng.integers(0, 2, size=n).astype(np.int32)
    print("loss:", kernel(b, s, y))



# revision 2
# speedup vs baseline: 1.1351x; 1.1351x over previous
"""Trainium2 Bass kernel for nn_Loss_20495583936604 (pairwise BCE ranking loss).

Reference semantics: over all pairs i<j with b[i]==b[j] and y[i]!=y[j],
mean of BCE-with-logits(d = s[i]-s[j], target z = (y[i]==1)).

Math reduction
--------------
Every valid unordered pair has exactly one positive (y==1) and one negative
(y==0) element, and its BCE term equals softplus(s_neg - s_pos) regardless of
index order.  So with segments g and P = sum_g |neg(g)|*|pos(g)| pairs:

    loss = (1/P) * sum_g sum_{n in neg(g)} sum_{p in pos(g)}
                       log(1 + exp(s_n) * exp(-s_p))

Host side does O(N) layout only: per segment, pack -s_pos into a [128, wp]
tile and s_neg into [128, wn] (partition = segment; NUM_SEGMENTS == 128),
padding with -1e4 so padded slots exp() to exactly 0 and contribute
log(1+0) = 0.  A trailing all-ones column rides along in the same DMA and
feeds the partition-reduce matmul (no gpsimd memset / extra semaphore).

Device side (one NeuronCore program, SPMD over 8 cores; cores split the
wn neg-slots — a data-parallel shard of the pair-matrix rows):
    1. one DMA brings in [-s_pos | s_neg-slice | 1.0]      (sync, HW DGE)
    2. e = exp(input)  - one ACT pass over both halves     (scalar)
    3. d = e_neg (x) e_pos outer product per partition via
       zero-stride broadcast APs - one DVE tensor_tensor   (vector)
    4. softplus = ln(d + 1) with free-dim accumulation     (scalar)
    5. partition reduce: ones^T @ acc matmul -> PSUM[1,1]  (tensor)
    6. PSUM -> SBUF copy, then a single-descriptor DMA out (vector+sync)
Host sums the 8 partial sums and divides by the (host-counted) pair count.

Perf notes baked in (vs the first working version, ~15.6us -> target <13us):
  * the semaphore-hygiene clears (dma_reset + sem_clear of the kernel sem
    range) are emitted DURING Bass.__init__, before the stock init
    all-engine barrier, so that single barrier orders both the clears and
    the const-AP memsets -- the separate ~1.1us NRT pseudo-barrier the
    first version needed after its post-init clears is gone entirely, and
    the input DMA issues ~1us earlier;
  * one explicit ACT table load of the combined "natural_log_exp_and_others"
    set (act_func_set_id=6) is emitted as the scalar engine's first
    instruction.  It covers BOTH the exp and the ln activations, so the
    ~1.3us natural_log table load that used to sit half-exposed between the
    DVE multiply and the ln pass is gone, as is the dummy-exp preload;
  * the all-ones vector for the partition-reduce matmul arrives as a 53rd
    column of the input DMA instead of a gpsimd memset + semaphore;
  * the output is reduced to [1,1] on-chip because a [128,1] store sprays
    128 4-byte descriptors over 16 DMA queues whose per-queue semaphore
    increments straggle in over ~5us;
  * the kernel ends with a sem-only barrier + semaphore clear so the core
    is left clean for the next NEFF (omitting this wedges the device).
"""

import sys

if "/opt/trn_rl_repo" not in sys.path:
    sys.path.insert(0, "/opt/trn_rl_repo")

import numpy as np

import concourse.bass as bass
from concourse import bacc, mybir
from concourse.bass_utils import run_bass_kernel_spmd

N_CORES = 8
N_PART = 128
PAD = -1.0e4  # exp(PAD) == 0.0 in f32
SCORE_RANGE_LIMIT = 25.0  # |s_i - s_j| beyond this risks exp/ln range issues
ACT_SET_LN_EXP = 6  # act_info.json index of "natural_log_exp_and_others"

_program_cache: dict[tuple[int, int], "bacc.Bacc"] = {}


def _build_program(wp: int, k: int) -> "bacc.Bacc":
    f32 = mybir.dt.float32
    w_tot = wp + k

    # Stock Bass.__init__ memsets four const APs and then runs an ALL-engine
    # barrier.  Patch the gpsimd memset hook so that (a) the kernel's
    # semaphore-hygiene clears (a prior NEFF may leave sems nonzero; waits
    # would then pass before their producers ran) land BEFORE that barrier,
    # letting the one stock barrier order everything -- no separate NRT
    # pseudo-barrier needed after init; and (b) only the two const APs this
    # kernel reads (f32 0.0 = exp bias, f32 1.0 = ln bias) are memset.
    orig_memset = bass.BassGpSimd.memset
    state = {"first": True}

    def patched_const_memset(self, ap, value, *args, **kwargs):
        name = getattr(ap.tensor, "name", "")
        if name.startswith("const-"):
            if state["first"]:
                state["first"] = False
                # block_sem (150) and the kernel sem range (153-255); the
                # barrier pair 151/152 must stay untouched (the imminent
                # init barrier uses it, and its protocol is self-cleaning).
                self.dma_reset(range(150, 151))
                self.sem_clear(range(150, 151))
                self.dma_reset(range(153, 256))
                self.sem_clear(range(153, 256))
            if name not in ("const-float32-0.0", "const-float32-1.0"):
                return None
        return orig_memset(self, ap, value, *args, **kwargs)

    bass.BassGpSimd.memset = patched_const_memset
    try:
        nc = bacc.Bacc(
            "TRN2", target_bir_lowering=False, debug=False, enable_asserts=False
        )
    finally:
        bass.BassGpSimd.memset = orig_memset

    inp = nc.dram_tensor("inp", [N_PART, w_tot + 1], f32, kind="ExternalInput")
    acc = nc.dram_tensor("acc", [1, 1], f32, kind="ExternalOutput")

    dma_sem = nc.alloc_semaphore("dma_sem")
    s_sem = nc.alloc_semaphore("s_sem")
    v_sem = nc.alloc_semaphore("v_sem")
    t_sem = nc.alloc_semaphore("t_sem")
    all_sems = [dma_sem, s_sem, v_sem, t_sem]
    # the init-time hygiene clear covered 153-255; all kernel sems must be in it
    assert all(153 <= h.num <= 255 for h in all_sems), [h.num for h in all_sems]

    with (
        nc.sbuf_tensor("in_t", [N_PART, w_tot + 1], f32) as in_t,
        nc.sbuf_tensor("e_t", [N_PART, w_tot], f32) as e_t,
        nc.sbuf_tensor("d_t", [N_PART, k * wp], f32) as d_t,
        nc.sbuf_tensor("sp_t", [N_PART, k * wp], f32) as sp_t,
        nc.sbuf_tensor("acc_t", [N_PART, 1], f32) as acc_t,
        nc.sbuf_tensor("red_t", [1, 1], f32) as red_t,
        nc.psum_tensor("psum_t", [1, 1], f32) as psum_t,
    ):
        e_ap = e_t.ap()
        a_neg = e_ap[:, wp : wp + k].unsqueeze(-1).broadcast_to([N_PART, k, wp])
        b_pos = e_ap[:, 0:wp].unsqueeze(1).broadcast_to([N_PART, k, wp])
        d3 = d_t.ap().rearrange("p (k w) -> p k w", k=k)

        # one table load covering exp AND ln, issued into the input-DMA
        # latency shadow; Bacc.insert_act_table_loads sees it dominating
        # both activations and inserts nothing further.
        nc.scalar.add_instruction(
            mybir.InstLoadActFuncSet(
                name=nc.get_next_instruction_name(),
                act_func_set_id=ACT_SET_LN_EXP,
                ins=[],
                outs=[],
            )
        )

        # input load (HW DGE): [-s_pos | s_neg-slice | ones]
        nc.sync.dma_start(in_t[:], inp.ap()).then_inc(dma_sem, 16)

        # e = exp(in): exp(-s_pos) | exp(s_neg) in one pass (ones col excluded)
        nc.scalar.wait_ge(dma_sem, 16)
        nc.scalar.activation(
            e_t[:], in_t[:, 0:w_tot], mybir.ActivationFunctionType.Exp
        ).then_inc(s_sem, 1)

        # all pairwise products exp(s_n)*exp(-s_p) via zero-stride broadcasts
        nc.vector.wait_ge(s_sem, 1)
        nc.vector.tensor_tensor(d3, a_neg, b_pos, op=mybir.AluOpType.mult).then_inc(
            v_sem, 1
        )

        # softplus = ln(d + 1), accumulated along the free dim
        nc.scalar.wait_ge(v_sem, 1)
        nc.scalar.activation(
            sp_t[:],
            d_t[:],
            mybir.ActivationFunctionType.Ln,
            bias=1.0,
            accum_out=acc_t[:],
        ).then_inc(s_sem, 1)

        # partition reduce on PE: psum[1,1] = acc^T @ ones (ones from the DMA)
        nc.tensor.wait_ge(s_sem, 2)
        nc.tensor.matmul(
            psum_t[:], acc_t[:], in_t[:, w_tot : w_tot + 1], start=True, stop=True
        ).then_inc(t_sem, 1)

        # PSUM -> SBUF on the idle vector engine, then one [1,1] descriptor
        nc.vector.wait_ge(t_sem, 1)
        nc.vector.tensor_copy(red_t[:], psum_t[:]).then_inc(v_sem, 1)

        nc.sync.wait_ge(v_sem, 2)
        nc.sync.dma_start(acc.ap(), red_t[:]).then_inc(dma_sem, 16)
        nc.sync.wait_ge(dma_sem, 32)

    # leave the core clean: sem-only barrier (dma_sem>=32 above already
    # confirmed every DMA completed), then gpsimd zeroes the kernel sems.
    nc.all_engine_barrier(sem_only=True)
    nc.clear_and_free_semaphores(all_sems)

    nc.compile()
    return nc


def pack(seg_ids, scores, width, pad):
    """Pack per-segment values into a [128, width] tile, pad-filled."""
    out = np.full((N_PART, width), pad, dtype=np.float32)
    order = np.argsort(seg_ids, kind="stable")
    sorted_seg = seg_ids[order]
    sorted_scores = scores[order]
    counts = np.bincount(sorted_seg, minlength=N_PART)
    starts = np.concatenate([[0], np.cumsum(counts)[:-1]])
    slot = np.arange(len(sorted_seg)) - starts[sorted_seg]
    out[sorted_seg, slot] = sorted_scores
    return out


def make_in_maps(b, s, y):
    seg = np.asarray(b).astype(np.int64)
    s = np.asarray(s, dtype=np.float32)
    is_pos = np.asarray(y) == 1
    cn = np.bincount(seg[~is_pos], minlength=N_PART).astype(np.int64)
    cp = np.bincount(seg[is_pos], minlength=N_PART).astype(np.int64)
    num_pairs = int((cn * cp).sum())
    if num_pairs == 0:
        return None, 0, 0, 0
    wn = int(-(-int(cn.max()) // N_CORES) * N_CORES)  # round up to 8 slots
    wp = int(cp.max())
    k = wn // N_CORES
    sn_packed = pack(seg[~is_pos], s[~is_pos], wn, PAD)
    nsp_packed = pack(seg[is_pos], -s[is_pos], wp, PAD)
    ones_col = np.ones((N_PART, 1), dtype=np.float32)
    in_maps = [
        {
            "inp": np.ascontiguousarray(
                np.concatenate(
                    [nsp_packed, sn_packed[:, c * k : (c + 1) * k], ones_col], axis=1
                )
            )
        }
        for c in range(N_CORES)
    ]
    return in_maps, num_pairs, wp, k


def _host_reference(seg, s, is_pos, num_pairs):
    """Exact fallback for inputs outside the device kernel's numeric
    envelope (never taken for the intended score distribution)."""
    total = 0.0
    for g in range(int(seg.max()) + 1):
        sn = s[(seg == g) & ~is_pos].astype(np.float64)
        sp = s[(seg == g) & is_pos].astype(np.float64)
        if len(sn) and len(sp):
            d = sn[:, None] - sp[None, :]
            total += np.logaddexp(0.0, d).sum()
    return np.float32(total / num_pairs)


def kernel(b: np.ndarray, s: np.ndarray, y: np.ndarray) -> np.ndarray:
    seg = np.asarray(b).astype(np.int64)
    s = np.asarray(s, dtype=np.float32)
    is_pos = np.asarray(y) == 1
    assert seg.min() >= 0 and seg.max() < N_PART, "segment ids must fit 128 partitions"

    in_maps, num_pairs, wp, k = make_in_maps(b, s, y)
    if num_pairs == 0:
        return np.float32(np.nan)
    if float(s.max()) - float(s.min()) > SCORE_RANGE_LIMIT:
        return _host_reference(seg, s, is_pos, num_pairs)

    key = (wp, k)
    nc = _program_cache.get(key)
    if nc is None:
        nc = _build_program(wp, k)
        _program_cache[key] = nc

    results = run_bass_kernel_spmd(nc, in_maps, core_ids=list(range(N_CORES))).results
    total = sum(np.float64(r["acc"][0, 0]) for r in results)
    if not np.isfinite(total):
        # device state was poisoned by a prior NEFF -- fall back to exact host math
        return _host_reference(seg, s, is_pos, num_pairs)
    return np.asarray(total / num_pairs, dtype=np.float32)


if __name__ == "__main__":
    rng = np.random.default_rng(0)
    n = 8192
    b = rng.integers(0, 128, size=n).astype(np.int32)
    s = rng.standard_normal(n).astype(np.float32)
    y = rng.integers(0, 2, size=n).astype(np.int32)
    print("loss:", kernel(b, s, y))
